# revision 26
# baseline (speedup 1.0000x reference)
"""DeepseekV3 FP8-block-dequant SwiGLU MLP on 8 TRN2 NeuronCores.

Computation: y = (silu(x @ dq(w_gate).T) * (x @ dq(w_up).T)) @ dq(w_down).T
with dq(w)[o,i] = w[o,i] * s[o//128, i//128].

Sharding: tensor-parallel over the F=2048 intermediate dim, 256 per core.
Each core computes a rank-256 partial of the output; partials are summed
on the host (the unshard step for a contraction-sharded output).

Device layout (prepared host-side, bf16):
  xp   [128, 56*512] : xp[p, k*512+t] = x[t, k*128+p]            (same on all cores)
  wgu  [128, 56*512] : wgu[p, k*512+m*128+f] = w_gate[c*256+m*128+f, k*128+p]; +256 up
  sgub [128, 56*512] : block-scale value for the matching wgu element (broadcast)
  wdp  [128, 2*7168] : wdp[p, k2*7168+h] = w_down[h, c*256+k2*128+p]
  sdb  [128, 2*7168] : block-scale value for the matching wdp element (broadcast)
  sgu  [128, 224]    : tiny fp32 grid, only used as PE-warmup matmul fodder
All matmuls contract over the partition dim. Dequant is elementwise
w *= scale done in-place in the weight landing buffers by single wide DVE
tensor-tensor ops (the scale tensors are host-side broadcasts of the given
16x56 / 56x16 scale grids - replication only, no host arithmetic on weights).
"""

import sys

if "/opt/trn_rl_repo" not in sys.path:
    sys.path.insert(0, "/opt/trn_rl_repo")

from contextlib import ExitStack

import ml_dtypes
import numpy as np

import concourse.bacc as bacc
import concourse.mybir as mybir
import concourse.tile as tile
from concourse import bass_utils

T, H, F = 512, 7168, 2048
NCORES = 8
FC = F // NCORES  # 256 intermediate channels per core
KT = H // 128  # 56 contraction k-tiles for gate/up
HN = H // 512  # 14 output column chunks for down matmul
BF16 = mybir.dt.bfloat16
F32 = mybir.dt.float32

_CACHE = {}


def _build_program(repeats=1):
    nc = bacc.Bacc("TRN2", target_bir_lowering=False, debug=False, num_devices=NCORES)

    xd = nc.dram_tensor("xp", [128, KT * T], BF16, kind="ExternalInput")
    wgud = nc.dram_tensor("wgu", [128, KT * 2 * FC], BF16, kind="ExternalInput")
    sgubd = nc.dram_tensor("sgub", [128, KT * 2 * FC], BF16, kind="ExternalInput")
    wdd = nc.dram_tensor("wdp", [128, 2 * H], BF16, kind="ExternalInput")
    sdbd = nc.dram_tensor("sdb", [128, 2 * H], BF16, kind="ExternalInput")
    sgud = nc.dram_tensor("sgu", [128, 4 * KT], F32, kind="ExternalInput")
    yd = nc.dram_tensor("y", [T, H], BF16, kind="ExternalOutput")

    with tile.TileContext(nc) as tc, ExitStack() as ctx:
        consts = ctx.enter_context(tc.tile_pool(name="consts", bufs=1))
        xpool = ctx.enter_context(tc.tile_pool(name="xpool", bufs=3))
        wpool = ctx.enter_context(tc.tile_pool(name="wpool", bufs=3))
        scpool = ctx.enter_context(tc.tile_pool(name="scpool", bufs=3))
        silpool = ctx.enter_context(tc.tile_pool(name="silpool", bufs=2))
        hpool = ctx.enter_context(tc.tile_pool(name="hpool", bufs=2))
        wdraw_pool = ctx.enter_context(tc.tile_pool(name="wdraw", bufs=2))
        sdb_pool = ctx.enter_context(tc.tile_pool(name="sdbp", bufs=2))
        ystage = ctx.enter_context(tc.tile_pool(name="ystage", bufs=2))
        pgu = ctx.enter_context(tc.tile_pool(name="pgu", bufs=4, space="PSUM"))
        pd = ctx.enter_context(tc.tile_pool(name="pd", bufs=4, space="PSUM"))

        sgu_sb = consts.tile([128, 4 * KT], F32, name="sgu_sb", tag="sgu_sb")
        nc.sync.dma_start(sgu_sb[:], sgud.ap())

        def emit_body():
            # ---- PE warmup: dummy fp32 matmuls on the (tiny, already-loaded)
            # scale grid keep the PE HAM activity window busy during the DMA
            # pipeline fill, so the real matmul stream starts at 2.4 GHz.
            ps_warm = pd.tile([128, 512], F32, name="ps_warm", tag="pd")
            for _ in range(16):
                nc.tensor.matmul(
                    ps_warm[:, : 2 * KT],
                    sgu_sb[:, :128],
                    sgu_sb[:, : 2 * KT],
                    start=True,
                    stop=True,
                )

            # ---- phase 1: gT/uT = dq(w).T @ x.T tiles, accumulated over 56 k
            psg = [pgu.tile([128, T], F32, name=f"psg{m}", tag="p1") for m in range(2)]
            psu = [pgu.tile([128, T], F32, name=f"psu{m}", tag="p1") for m in range(2)]

            # small first chunks so PE starts early; small last chunks so the
            # post-DMA compute tail is short
            chunks = [2, 2, 4, 8, 8, 8, 8, 8, 4, 2, 2]
            assert sum(chunks) == KT

            k = 0
            for g, cs in enumerate(chunks):
                nb = {2: 2, 4: 2, 8: 3}[cs]
                cols = slice(k * T, (k + cs) * T)
                xc = xpool.tile(
                    [128, cs * T], BF16, name=f"xc{g}", tag=f"xc{cs}", bufs=nb
                )
                nc.sync.dma_start(xc[:], xd.ap()[:, cols])
                wc = wpool.tile(
                    [128, cs * 2 * FC], BF16, name=f"wc{g}", tag=f"wc{cs}", bufs=nb
                )
                nc.sync.dma_start(wc[:], wgud.ap()[:, cols])
                sc = scpool.tile(
                    [128, cs * 2 * FC],
                    BF16,
                    name=f"sc{g}",
                    tag=f"sc{cs}",
                    bufs={2: 2, 4: 2, 8: 2}[cs],
                )
                nc.sync.dma_start(sc[:], sgubd.ap()[:, cols])
                # in-place dequant of the whole chunk in one DVE op
                nc.vector.tensor_mul(wc[:], wc[:], sc[:])
                for j in range(cs):
                    start, stop = (k == 0), (k == KT - 1)
                    rhs = xc[:, j * T : (j + 1) * T]
                    for m in range(2):
                        nc.tensor.matmul(
                            psg[m][:],
                            wc[:, j * 512 + m * 128 : j * 512 + (m + 1) * 128],
                            rhs,
                            start=start,
                            stop=stop,
                        )
                        nc.tensor.matmul(
                            psu[m][:],
                            wc[:, j * 512 + 256 + m * 128 : j * 512 + 256 + (m + 1) * 128],
                            rhs,
                            start=start,
                            stop=stop,
                        )
                    k += 1

            # ---- down-proj weights + scales: issued after all gate/up traffic
            # (phase-2 compute overlaps these loads). Half-tensor DMAs ordered
            # to match the n-major in-place dequant below.
            wdr = [
                wdraw_pool.tile([128, H], BF16, name=f"wdr{i}", tag="wdr")
                for i in range(2)
            ]
            sdb = [
                sdb_pool.tile([128, H], BF16, name=f"sdb{i}", tag="sdb")
                for i in range(2)
            ]
            for half in range(2):
                lo, hi = half * (H // 2), (half + 1) * (H // 2)
                for k2 in range(2):
                    nc.sync.dma_start(
                        wdr[k2][:, lo:hi], wdd.ap()[:, k2 * H + lo : k2 * H + hi]
                    )
                    nc.sync.dma_start(
                        sdb[k2][:, lo:hi], sdbd.ap()[:, k2 * H + lo : k2 * H + hi]
                    )

            # ---- h = silu(g) * u = sigmoid(g) * g * u, in [128, 128] column
            # slices so phase 2's t=0 matmuls can start early
            sil = [
                silpool.tile([128, T], BF16, name=f"sil{m}", tag="sil")
                for m in range(2)
            ]
            tmp = [
                silpool.tile([128, T], BF16, name=f"sgm{m}", tag="sgm")
                for m in range(2)
            ]
            hts = [
                hpool.tile([128, T], BF16, name=f"ht{m}", tag="ht") for m in range(2)
            ]
            for tt in range(4):
                sl = slice(tt * 128, (tt + 1) * 128)
                for m in range(2):
                    # sigmoid + two muls rather than Silu: CoreSim lacks Silu,
                    # and on HW the Silu LUT measured ~15us slower end-to-end
                    nc.scalar.activation(
                        sil[m][:, sl],
                        psg[m][:, sl],
                        mybir.ActivationFunctionType.Sigmoid,
                    )
                    nc.vector.tensor_mul(tmp[m][:, sl], sil[m][:, sl], psg[m][:, sl])
                    nc.vector.tensor_mul(hts[m][:, sl], tmp[m][:, sl], psu[m][:, sl])

            # ---- phase 2 in-place dequant (n-major so early n chunks are
            # ready first), then y_partial[t, h] = hT.T @ dq(wd) over f
            for n in range(HN):
                for k2 in range(2):
                    cs = slice(n * 512, (n + 1) * 512)
                    nc.vector.tensor_mul(wdr[k2][:, cs], wdr[k2][:, cs], sdb[k2][:, cs])

            for t in range(4):
                for half in range(2):
                    ystg = ystage.tile([128, H // 2], BF16, name=f"ys{t}{half}", tag="ys")
                    for nh in range(HN // 2):
                        n = half * (HN // 2) + nh
                        ps = pd.tile([128, 512], F32, name=f"ps{t}_{n}", tag="pd")
                        for k2 in range(2):
                            nc.tensor.matmul(
                                ps[:],
                                hts[k2][:, t * 128 : (t + 1) * 128],
                                wdr[k2][:, n * 512 : (n + 1) * 512],
                                start=(k2 == 0),
                                stop=(k2 == 1),
                            )
                        dst = ystg[:, nh * 512 : (nh + 1) * 512]
                        if (n + t) % 4 == 0:
                            nc.vector.tensor_copy(dst, ps[:])
                        else:
                            nc.scalar.copy(dst, ps[:])
                    nc.sync.dma_start(
                        yd.ap()[
                            t * 128 : (t + 1) * 128,
                            half * (H // 2) : (half + 1) * (H // 2),
                        ],
                        ystg[:],
                    )

        for _rep in range(repeats):
            emit_body()

    nc.compile()
    return nc


def _get_program():
    if "nc" not in _CACHE:
        _CACHE["nc"] = _build_program()
    return _CACHE["nc"]


def _prep_inputs(x, w_gate, s_gate, w_up, s_up, w_down, s_down):
    bf = ml_dtypes.bfloat16
    # x -> [p, k, t] -> [128, KT*T]
    xp = np.ascontiguousarray(
        x.reshape(T, KT, 128).transpose(2, 1, 0).reshape(128, KT * T)
    ).astype(bf)
    in_maps = []
    for c in range(NCORES):
        gsl = slice(c * FC, (c + 1) * FC)
        ag = w_gate[gsl].reshape(FC, KT, 128).transpose(2, 1, 0)  # [p, k, f]
        au = w_up[gsl].reshape(FC, KT, 128).transpose(2, 1, 0)
        wgu = np.ascontiguousarray(
            np.concatenate([ag, au], axis=2).reshape(128, KT * 2 * FC)
        ).astype(bf)
        # scale rows matching wgu's [k, 4x128] column layout, broadcast to
        # all 128 partitions (scale blocks are 128x128, so within one k-tile
        # the scale is constant across partitions and per 128-col group)
        srow = np.empty((KT, 4, 128), np.float32)
        srow[:, 0, :] = s_gate[2 * c][:, None]
        srow[:, 1, :] = s_gate[2 * c + 1][:, None]
        srow[:, 2, :] = s_up[2 * c][:, None]
        srow[:, 3, :] = s_up[2 * c + 1][:, None]
        sgub = np.ascontiguousarray(
            np.broadcast_to(
                srow.reshape(1, KT * 2 * FC).astype(bf), (128, KT * 2 * FC)
            )
        )
        wdp = np.ascontiguousarray(
            w_down[:, gsl].reshape(H, 2, 128).transpose(2, 1, 0).reshape(128, 2 * H)
        ).astype(bf)
        drow = np.empty((2, KT, 128), np.float32)
        drow[0] = s_down[:, 2 * c][:, None]
        drow[1] = s_down[:, 2 * c + 1][:, None]
        sdb = np.ascontiguousarray(
            np.broadcast_to(drow.reshape(1, 2 * H).astype(bf), (128, 2 * H))
        )
        sgu = np.ascontiguousarray(
            np.broadcast_to(
                np.concatenate(
                    [s_gate[2 * c], s_gate[2 * c + 1], s_up[2 * c], s_up[2 * c + 1]]
                ).astype(np.float32),
                (128, 4 * KT),
            )
        )
        in_maps.append(
            {"xp": xp, "wgu": wgu, "sgub": sgub, "wdp": wdp, "sdb": sdb, "sgu": sgu}
        )
    return in_maps


def kernel(x, w_gate, s_gate, w_up, s_up, w_down, s_down, _trace=False):
    x = np.asarray(x, np.float32)
    w_gate = np.asarray(w_gate, np.float32)
    w_up = np.asarray(w_up, np.float32)
    w_down = np.asarray(w_down, np.float32)
    s_gate = np.asarray(s_gate, np.float32)
    s_up = np.asarray(s_up, np.float32)
    s_down = np.asarray(s_down, np.float32)

    nc = _get_program()
    in_maps = _prep_inputs(x, w_gate, s_gate, w_up, s_up, w_down, s_down)
    res = bass_utils.run_bass_kernel_spmd(
        nc, in_maps, core_ids=list(range(NCORES)), trace=_trace
    )
    y = np.zeros((T, H), np.float32)
    for c in range(NCORES):
        y += res.results[c]["y"].astype(np.float32)
    if _trace:
        _CACHE["last_results"] = res
    return y
